# revision 4
# baseline (speedup 1.0000x reference)
"""Multi-head attention (B=4, S=2048, D=1024, H=16, Hd=64) on 8 TRN2 NeuronCores.

Sharding: tensor-parallel over heads - 2 heads per core (128 channels).
Each core computes its heads' Q/K/V projections, attention, and the partial
output projection (its 128 rows of Wo); the host sums the 8 partials + bo.

Bias algebra (exact):
  - K bias cancels in softmax, never applied.
  - V bias commutes with the attention average: bv @ Wo added on host.
  - Only the Q bias is applied on device.

Pipeline (vs the ~460us single-score-tile baseline; measured ~382us at full
clock, ~455us when the chip is in the P0 power-state downclock):
  - The softmax exp is split into two 2-bank ACT ops per super-step (one per
    k-tile parity), with per-parity score PSUM tiles allocated from a bufs=2
    ring and per-parity p tiles. The tile framework tracks deps per tile, so
    scores(s+1, even) only waits on expA(s) - the baseline's 2.6us/sup
    serialization cycle (exp -> scores -> exp on one 4-bank tile) becomes a
    free-running pipeline where ACT runs back-to-back.
  - Emission order per sup:
      scores(even) | expA | AV(prev, even) | scores(odd) | expB | AV(prev, odd)
    AV is kt-major so the even AV pair depends only on expA.
  - qt/kt/v are per-batch tiles so projection-copy (DVE) -> scores (PE) RAW
    deps cannot reach across batches (dep tracking is per-tile).
  - Projections are pulled at a smooth ~0.63 units/sup rate along the exact
    per-batch deadline envelope (no 29us bursts at batch boundaries).
  - The previous block's output projection runs at the top of sups 3/5/6/7,
    so its psW tiles WAR against pulls a full sup back and the norm chain
    (GpSimd broadcast + reciprocal, started at s==0) is long done.
  - The final block's finalize is split into per-j chains with PSUM
    evacuation on the (then idle) Scalar engine, and the softmax reciprocal
    runs on the compact [1,512] denominator rows BEFORE the GpSimd
    broadcast, so the drain's serial chain is minimal.
  - Host pre-arranges x and the weights so every DMA is per-partition
    contiguous (the baseline's rearranging DMAs serialized ~16us of
    descriptor generation on the Sync engine before the first matmul).
  - y partials are written in bf16 (the host accumulates in fp64).
"""
import sys

sys.path.insert(0, "/opt/trn_rl_repo")

import numpy as np
import ml_dtypes

import concourse.bass as bass
import concourse.mybir as mybir
import concourse.tile as tile
from concourse import bacc, bass_utils
from concourse.masks import make_identity

B, S, D = 4, 2048, 1024
BS = B * S            # 8192 rows
NCORES = 8
CPC = 128             # channels per core (2 heads x 64)
HD = 64               # head dim
P = 128
QT = 512              # q-tile width
NKT = S // P          # 16 k-tiles per batch
NSUP = NKT // 2       # 8 super-steps (2 k-tiles each)
NLQ = BS // QT        # 16 global q-tiles
KCH = D // P          # 8 contraction chunks for the projections

F32 = mybir.dt.float32
CD = mybir.dt.bfloat16
CD_NP = ml_dtypes.bfloat16

LAST_RESULTS = None
_NC_CACHE = {}

# projection generator unit positions (1-based pulled-counts), 5 per q-tile:
#   5*lq+1 dma, +2 Q, +3 K, +4 V, +5 Vt
UNITS_PER_LQ = 5
TOTAL_UNITS = UNITS_PER_LQ * NLQ  # 80


def build_nc():
    if "nc" in _NC_CACHE:
        return _NC_CACHE["nc"]
    nc = bacc.Bacc(trn_type="TRN2", num_devices=NCORES)

    # host-prearranged layouts: everything per-partition contiguous
    xh = nc.dram_tensor("xh", [NLQ, P, KCH, QT], CD, kind="ExternalInput").ap()
    wq = nc.dram_tensor("wq", [P, KCH, CPC], CD, kind="ExternalInput").ap()
    wk = nc.dram_tensor("wk", [P, KCH, CPC], CD, kind="ExternalInput").ap()
    wv = nc.dram_tensor("wv", [P, KCH, CPC], CD, kind="ExternalInput").ap()
    wo = nc.dram_tensor("wo", [CPC, D], CD, kind="ExternalInput").ap()
    bq = nc.dram_tensor("bq", [CPC, 1], F32, kind="ExternalInput").ap()
    y = nc.dram_tensor("y", [BS, D], CD, kind="ExternalOutput").ap()

    scale = float(1.0 / np.sqrt(np.float32(HD)))

    with tile.TileContext(nc) as tc:
        with (
            tc.tile_pool(name="pers", bufs=1) as pers,
            tc.tile_pool(name="xin", bufs=5) as xin,
            tc.tile_pool(name="vtp", bufs=2) as vtp,
            tc.tile_pool(name="pt", bufs=4) as pt,
            tc.tile_pool(name="otn", bufs=2) as otn_pool,
            tc.tile_pool(name="yp", bufs=3) as yp,
            tc.tile_pool(name="sm", bufs=6) as sm,
            tc.tile_pool(name="smd", bufs=6) as smd,
            tc.tile_pool(name="otu", bufs=6) as otu_pool,
            tc.tile_pool(name="psSC", bufs=2, space="PSUM") as psSC,
            tc.tile_pool(name="psW", bufs=2, space="PSUM") as psW,
            tc.tile_pool(name="psOT", bufs=2, space="PSUM") as psOT,
        ):
            # ---- persistent tensors (per-batch where attention reads them,
            # so cross-engine RAW deps stay batch-local) ----
            qt_sbs = [pers.tile([P, S], CD, tag=f"QT{b}", name=f"qt{b}") for b in range(B)]
            kt_sbs = [pers.tile([P, S], CD, tag=f"KT{b}", name=f"kt{b}") for b in range(B)]
            # per seq-tile, per head: [V_h | ones] so OT row 64 is the denom
            v_sbs = [
                pers.tile([P, NKT, 2, HD + 1], CD, tag=f"V{b}", name=f"v{b}") for b in range(B)
            ]
            wq_sb = pers.tile([P, KCH, CPC], CD, tag="wq")
            wk_sb = pers.tile([P, KCH, CPC], CD, tag="wk")
            wv_sb = pers.tile([P, KCH, CPC], CD, tag="wv")
            wo_sb = pers.tile([P, D], CD, tag="wo")
            bq_sb = pers.tile([CPC, 1], F32, tag="bq")
            ident_sb = pers.tile([P, P], CD, tag="ident")

            # ---- projections as a lazily-driven generator (PE filler) ----
            # Batch 0 runs K/V-first with its three later-block Q units
            # trailing (they're not needed until blocks (0,1..3)), so the
            # ramp sheds ~5us of PE work before the pipeline reaches steady
            # state. Batches 1-3 keep the plain per-q-tile order.
            def run_proj(pb, plq, which, xt):
                q0 = plq * QT
                pj = psW.tile([P, QT], F32, tag="w", name="pj")
                w_sb = {"q": wq_sb, "k": wk_sb, "v": wv_sb}[which]
                for o in range(KCH):
                    nc.tensor.matmul(
                        pj[:], w_sb[:, o, :], xt[:, o, :],
                        start=(o == 0), stop=(o == KCH - 1),
                    )
                if which == "q":
                    nc.vector.tensor_scalar_add(
                        qt_sbs[pb][:, q0 : q0 + QT], pj[:], bq_sb[:, 0:1]
                    )
                    return None
                if which == "k":
                    nc.vector.tensor_copy(
                        out=kt_sbs[pb][:, q0 : q0 + QT], in_=pj[:]
                    )
                    return None
                vt_sb = vtp.tile([P, QT], CD, tag="vt")
                nc.vector.tensor_copy(out=vt_sb[:], in_=pj[:])
                return vt_sb

            def run_vt(pb, plq, vt_sb):
                for rt in range(QT // P):
                    tp = psW.tile([P, P], CD, tag="w", name="tp")
                    nc.tensor.transpose(
                        tp[:], vt_sb[:, rt * P : (rt + 1) * P], ident_sb[:]
                    )
                    grt = plq * (QT // P) + rt
                    nc.vector.tensor_copy(
                        out=v_sbs[pb][:, grt, :, 0:HD],
                        in_=tp[:, :].rearrange("p (h c) -> p h c", h=2),
                    )

            def load_xt(lq):
                xt = xin.tile([P, KCH, QT], CD, tag="xt")
                if lq == 0:
                    h = KCH // 2
                    nc.sync.dma_start(xt[:, 0:h, :], xh[lq, :, 0:h, :])
                    nc.sync.dma_start(xt[:, h:KCH, :], xh[lq, :, h:KCH, :])
                else:
                    nc.sync.dma_start(xt[:], xh[lq, :, :, :])
                return xt

            def proj_gen():
                # batch 0: d0 Q0 K0 V0 Vt0 | d1 K1 V1 Vt1 | d2 ... | d3 ...
                #          | Q1 Q2 Q3
                xts = {}
                for plq in range(4):
                    xts[plq] = load_xt(plq)
                    yield
                    if plq == 0:
                        run_proj(0, 0, "q", xts[0])
                        yield
                    run_proj(0, plq, "k", xts[plq])
                    yield
                    vt_sb = run_proj(0, plq, "v", xts[plq])
                    yield
                    run_vt(0, plq, vt_sb)
                    yield
                for plq in range(1, 4):
                    run_proj(0, plq, "q", xts[plq])
                    yield
                # batches 1-3: plain per-q-tile order
                for pb in range(1, B):
                    for plq in range(4):
                        xt = load_xt(4 * pb + plq)
                        yield
                        run_proj(pb, plq, "q", xt)
                        yield
                        run_proj(pb, plq, "k", xt)
                        yield
                        vt_sb = run_proj(pb, plq, "v", xt)
                        yield
                        run_vt(pb, plq, vt_sb)
                        yield

            gen = proj_gen()
            pulled = [0]

            def pull_to(target):
                while pulled[0] < min(target, TOTAL_UNITS):
                    if next(gen, "done") == "done":
                        break
                    pulled[0] += 1

            # DMA order: wq + bq, then xh[0] (via the generator's first unit),
            # then the rest - so the first projection run starts ~7us sooner.
            nc.sync.dma_start(wq_sb[:], wq[:, :, :])
            nc.sync.dma_start(bq_sb[:], bq[:, :])
            pull_to(1)
            nc.sync.dma_start(wk_sb[:], wk[:, :, :])
            nc.sync.dma_start(wv_sb[:], wv[:, :, :])
            nc.sync.dma_start(wo_sb[:], wo[:, :])
            for b in range(B):
                nc.vector.memset(v_sbs[b][:, :, :, HD : HD + 1], 1.0)
            make_identity(nc, ident_sb[:])

            # batch-local 1-based unit positions (batch 0 is reordered)
            POS0_K = {0: 3, 1: 7, 2: 11, 3: 15}
            POS0_VT = {0: 5, 1: 9, 2: 13, 3: 17}
            POS0_Q = {0: 2, 1: 18, 2: 19, 3: 20}

            def req_units(b, qa, s):
                """deadline: units needed before emitting sup s of block."""
                lqK = (2 * s + 1) // 4
                lqV = (2 * s - 1) // 4
                if b == 0:
                    r = max(POS0_Q[qa], POS0_K[lqK])
                    if s >= 1:
                        r = max(r, POS0_VT[lqV])
                    return r
                base = 20 * b
                r = base + 5 * qa + 2
                r = max(r, base + 5 * lqK + 3)
                if s >= 1:
                    r = max(r, base + 5 * lqV + 5)
                return r

            # ---- attention ----
            def emit_scores(sc_t, b, qa, kt):
                q0 = qa * QT
                k0 = kt * P
                for h in range(2):
                    hp = h * HD
                    nc.tensor.matmul(
                        sc_t[:, h, :],
                        kt_sbs[b][hp : hp + HD, k0 : k0 + P],
                        qt_sbs[b][hp : hp + HD, q0 : q0 + QT],
                        start=True, stop=True,
                    )

            def emit_av_half(ot, b, kt, p_half, avst):
                """AV for k-tile kt of both heads from the parity p tile."""
                for h in range(2):
                    nc.tensor.matmul(
                        ot[h][0 : HD + 1, :],
                        v_sbs[b][:, kt, h, :],
                        p_half[:, h, :],
                        start=(avst[h] == 0), stop=(avst[h] == NKT - 1),
                    )
                    avst[h] += 1

            def evacuate(b, qa, ot):
                otu = [
                    otu_pool.tile([HD, QT], CD, tag="otu", name=f"otu{h}")
                    for h in range(2)
                ]
                dn = [
                    smd.tile([1, QT], F32, tag="dn", name=f"dn{h}")
                    for h in range(2)
                ]
                for h in range(2):
                    nc.vector.tensor_copy(out=otu[h][:], in_=ot[h][0:HD, :])
                    nc.vector.tensor_copy(out=dn[h][:], in_=ot[h][HD : HD + 1, :])
                return (b, qa, otu, dn)

            def finalize_norm(fin):
                b, qa, otu, dn = fin
                brd = sm.tile([HD, 2, QT], F32, tag="brd")
                for h in range(2):
                    nc.gpsimd.partition_broadcast(brd[:, h, :], dn[h][0:1, :])
                r_sb = sm.tile([HD, 2, QT], F32, tag="rsb")
                nc.vector.reciprocal_approx_fast(out=r_sb[:], in_=brd[:])
                on = otn_pool.tile([P, QT], CD, tag="otn")
                nc.vector.tensor_mul(
                    out=on[0:HD, :], in0=otu[0][:], in1=r_sb[:, 0, :]
                )
                nc.vector.tensor_mul(
                    out=on[HD:CPC, :], in0=otu[1][:], in1=r_sb[:, 1, :]
                )
                return (b, qa, on)

            def finalize_proj_j(norm, j):
                b, qa, on = norm
                q0 = b * S + qa * QT
                ysb = yp.tile([P, D], CD, tag="y")
                for e in range(D // QT):
                    yps = psW.tile([P, QT], F32, tag="w", name="yps")
                    nc.tensor.matmul(
                        yps[:],
                        on[:, j * P : (j + 1) * P],
                        wo_sb[:, e * QT : (e + 1) * QT],
                        start=True, stop=True,
                    )
                    nc.vector.tensor_copy(
                        out=ysb[:, e * QT : (e + 1) * QT], in_=yps[:]
                    )
                nc.sync.dma_start(y[q0 + j * P : q0 + (j + 1) * P, :], ysb[:])

            blocks = [(b, qa) for b in range(B) for qa in range(4)]
            carry = None     # (ot, b, qa, pA, pB, avst)
            pending = None   # evacuated, waiting norm
            norm_cur = None  # norm result being outproj'd
            for bi, (b, qa) in enumerate(blocks):
                ot = [
                    psOT.tile([P, QT], F32, tag="ot", name=f"ot{h}")
                    for h in range(2)
                ]
                avst = [0, 0]
                pA_prev = pB_prev = None
                for s in range(NSUP):
                    gs = bi * NSUP + s
                    # outproj of the previous block at the top of the sup:
                    # its psW tiles then WAR against pulls a full sup back,
                    # and the norm chain (started at s==0) is long done by s=3
                    if norm_cur is not None and s in (3, 5, 6, 7):
                        finalize_proj_j(norm_cur, {3: 0, 5: 1, 6: 2, 7: 3}[s])
                    if gs < NSUP:
                        # block (0,0): K/V-first ramp, trailing Qs excluded
                        quota = 3 + 2 * gs
                    else:
                        # rate 20 units / 32 sups; this line passes exactly
                        # through every batch's (b,0) deadline stairs
                        quota = 22 + ((gs - NSUP) * 20 + 31) // 32
                    pull_to(max(req_units(b, qa, s), quota))
                    # even k-tile: scores -> expA -> AV(prev sup, even)
                    scE = psSC.tile([P, 2, QT], F32, tag="sc", name="scE")
                    emit_scores(scE, b, qa, 2 * s)
                    pA = pt.tile([P, 2, QT], CD, tag="p", name="pA")
                    nc.scalar.activation(
                        pA[:], scE[:], mybir.ActivationFunctionType.Exp,
                        scale=scale,
                    )
                    if s == 0:
                        if carry is not None:
                            cot, cb, cqa, cpA, cpB, cavst = carry
                            emit_av_half(cot, cb, NKT - 2, cpA, cavst)
                    else:
                        emit_av_half(ot, b, 2 * (s - 1), pA_prev, avst)
                    # odd k-tile: scores -> expB -> AV(prev sup, odd)
                    scO = psSC.tile([P, 2, QT], F32, tag="sc", name="scO")
                    emit_scores(scO, b, qa, 2 * s + 1)
                    pB = pt.tile([P, 2, QT], CD, tag="p", name="pB")
                    nc.scalar.activation(
                        pB[:], scO[:], mybir.ActivationFunctionType.Exp,
                        scale=scale,
                    )
                    if s == 0:
                        if carry is not None:
                            emit_av_half(cot, cb, NKT - 1, cpB, cavst)
                            pending = evacuate(cb, cqa, cot)
                            carry = None
                        if pending is not None:
                            norm_cur = finalize_norm(pending)
                            pending = None
                    else:
                        emit_av_half(ot, b, 2 * (s - 1) + 1, pB_prev, avst)
                    pA_prev, pB_prev = pA, pB
                carry = (ot, b, qa, pA_prev, pB_prev, avst)
            # drain: the final block's finalize is split into per-j chains so
            # the DVE/GpSimd/PE/DMA stages pipeline instead of running the
            # whole-block chain serially after the last exp.
            cot, cb, cqa, cpA, cpB, cavst = carry
            emit_av_half(cot, cb, NKT - 2, cpA, cavst)
            emit_av_half(cot, cb, NKT - 1, cpB, cavst)
            # reciprocal on the compact [2, 512] denominator rows FIRST (two
            # DVE lanes), then one whole-width GpSimd broadcast per head -
            # shortens the drain's serial chain vs broadcast-then-reciprocal
            dn = [
                smd.tile([1, QT], F32, tag="dn", name=f"dnL{h}")
                for h in range(2)
            ]
            rr = [
                smd.tile([1, QT], F32, tag="dn", name=f"rrL{h}")
                for h in range(2)
            ]
            r_sb = sm.tile([HD, 2, QT], F32, tag="rsb", name="rL")
            for h in range(2):
                nc.vector.tensor_copy(
                    out=dn[h][:], in_=cot[h][HD : HD + 1, :]
                )
                nc.vector.reciprocal_approx_fast(out=rr[h][:], in_=dn[h][:])
                nc.gpsimd.partition_broadcast(r_sb[:, h, :], rr[h][0:1, :])
            q0 = cb * S + cqa * QT
            for j in range(QT // P):
                jP = j * P
                otuj = [
                    otu_pool.tile([HD, P], CD, tag="otu", name=f"otuL{h}{j}")
                    for h in range(2)
                ]
                for h in range(2):
                    # ACT is idle after the final exp: evacuate there
                    nc.scalar.copy(
                        out=otuj[h][:], in_=cot[h][0:HD, jP : jP + P]
                    )
                onj = otn_pool.tile([P, P], CD, tag="otn", name=f"onL{j}")
                nc.vector.tensor_mul(
                    out=onj[0:HD, :], in0=otuj[0][:], in1=r_sb[:, 0, jP : jP + P]
                )
                nc.vector.tensor_mul(
                    out=onj[HD:CPC, :], in0=otuj[1][:], in1=r_sb[:, 1, jP : jP + P]
                )
                ysb = yp.tile([P, D], CD, tag="y", name=f"yL{j}")
                for e in range(D // QT):
                    yps = psW.tile([P, QT], F32, tag="w", name="ypsL")
                    nc.tensor.matmul(
                        yps[:],
                        onj[:, :],
                        wo_sb[:, e * QT : (e + 1) * QT],
                        start=True, stop=True,
                    )
                    nc.scalar.copy(
                        out=ysb[:, e * QT : (e + 1) * QT], in_=yps[:]
                    )
                nc.sync.dma_start(y[q0 + jP : q0 + jP + P, :], ysb[:])

    nc.compile()
    _NC_CACHE["nc"] = nc
    return nc


def make_in_maps(inputs):
    x = np.asarray(inputs["x"], np.float32)
    Wq = np.asarray(inputs["Wq"], np.float32)
    Wk = np.asarray(inputs["Wk"], np.float32)
    Wv = np.asarray(inputs["Wv"], np.float32)
    Wo = np.asarray(inputs["Wo"], np.float32)
    bq = np.asarray(inputs["bq"], np.float32)

    # xh[lq, p, o, q] = x.reshape(BS, D).T[o*128+p, lq*512+q]
    xT = x.reshape(BS, D).T.astype(CD_NP)          # [D, BS]
    xh = np.ascontiguousarray(
        xT.reshape(KCH, P, NLQ, QT).transpose(2, 1, 0, 3)
    )

    def warr(W, sl):
        # [D, 128] -> [p, o, c]
        return np.ascontiguousarray(
            W[:, sl].astype(CD_NP).reshape(KCH, P, CPC).transpose(1, 0, 2)
        )

    in_maps = []
    for c in range(NCORES):
        sl = slice(c * CPC, (c + 1) * CPC)
        in_maps.append(
            {
                "xh": xh,
                "wq": warr(Wq, sl),
                "wk": warr(Wk, sl),
                "wv": warr(Wv, sl),
                "wo": np.ascontiguousarray(Wo[sl, :]).astype(CD_NP),
                "bq": np.ascontiguousarray(bq[sl].reshape(CPC, 1)),
            }
        )
    return in_maps


def kernel(**inputs):
    global LAST_RESULTS
    bo = np.asarray(inputs["bo"], np.float64)
    bv = np.asarray(inputs["bv"], np.float64)
    Wo = np.asarray(inputs["Wo"], np.float64)
    nc = build_nc()
    in_maps = make_in_maps(inputs)
    res = bass_utils.run_bass_kernel_spmd(nc, in_maps, core_ids=list(range(NCORES)))
    LAST_RESULTS = res
    acc = np.zeros((BS, D), np.float64)
    for r in res.results:
        acc += np.asarray(r["y"]).astype(np.float64)
    out = (acc + bo + bv @ Wo).astype(np.float32)
    return out.reshape(B, S, D)
